# revision 1
# baseline (speedup 1.0000x reference)
"""Trainium2 Bass kernel for nn_Encoder_61177514164477 (meta-GCN LSTM encoder).

Sharding: 8 cores = 4 batch groups x 2 node-halves. Core c handles batch
b = c//2 and node rows [half*1024, (half+1)*1024) with half = c%2.
G^T (bf16) stays SBUF-resident per core; one pairwise AllGather per timestep
exchanges the h states between the two halves of each batch pair.

Host side precomputes the (tiny) meta-learner MLPs (W_t, bias_t per layer)
and lays every tensor out in device-friendly partition-major order, with the
node axis permuted per core to local order [own-half, partner-half].
"""
import os
import numpy as np
import ml_dtypes

NO_GPSIMD = os.environ.get("NO_GPSIMD", "") != ""
STATIC_PARTNER = os.environ.get("STATIC_PARTNER", "") != ""
NO_CC = os.environ.get("NO_CC", "") != ""

import concourse.bass as bass
import concourse.mybir as mybir
import concourse.tile as tile
import concourse.bacc as bacc
import concourse.tile_utils as tile_utils
from concourse.bass_utils import run_bass_kernel_spmd

# use the full cayman SBUF (224 KiB phys / ~208 usable per partition)
tile_utils.max_sbuf_usage = 204 * 1024

L, B, T, N, C, H, K, M = 2, 4, 8, 2048, 2, 64, 3, 32
DIN0, DIN1, DOUT = C + H, 2 * H, 4 * H
HALF = N // 2          # 1024 rows per core
JT = N // 128          # 16 j-tiles (local order: 8 own + 8 partner)
IT = HALF // 128       # 8 own i-tiles
NCORES = 8
PAIRS = [[0, 1], [2, 3], [4, 5], [6, 7]]

F32 = mybir.dt.float32
BF16 = mybir.dt.bfloat16
I32 = mybir.dt.int32
BF = ml_dtypes.bfloat16

SC0 = DIN0             # 66 stationary cols per j-tile, layer 0 ([h(64)|x(2)])
SC1 = DIN1             # 128 stationary cols per j-tile, layer 1 ([h0|h1])
D0R = DIN0 + 1         # 67 supT rows for layer 0 (incl. ones row for bias)

_CACHE = {}
LAST_RESULT = None


def _build():
    if "nc" in _CACHE:
        return _CACHE["nc"]
    nc = bacc.Bacc(None, target_bir_lowering=False, debug=False)

    gt_in = nc.declare_dram_parameter("gt", [K, JT, 128, HALF], BF16, isOutput=False)
    x_in = nc.declare_dram_parameter("x_all", [128, JT * T * C], BF16, isOutput=False)
    s0i_in = nc.declare_dram_parameter("stat0_init", [128, JT * SC0], BF16, isOutput=False)
    s1i_in = nc.declare_dram_parameter("stat1_init", [128, JT * SC1], BF16, isOutput=False)
    c0_in = nc.declare_dram_parameter("c0_init", [128, IT * H], F32, isOutput=False)
    c1_in = nc.declare_dram_parameter("c1_init", [128, IT * H], F32, isOutput=False)
    h1b_in = nc.declare_dram_parameter("h1b_init", [128, IT * H], BF16, isOutput=False)
    w0_in = nc.declare_dram_parameter("w0", [D0R, T * K * DOUT], BF16, isOutput=False)
    w1_in = nc.declare_dram_parameter("w1", [SC1, T * K * DOUT], BF16, isOutput=False)
    b1_in = nc.declare_dram_parameter("bias1", [128, T * DOUT], F32, isOutput=False)
    mask_in = nc.declare_dram_parameter("mask", [128, 2], F32, isOutput=False)
    out_ext = nc.declare_dram_parameter("out", [2, L, 128, IT * H], F32, isOutput=True)

    def GP_COPY(dst, src):
        if NO_GPSIMD:
            nc.vector.tensor_copy(dst, src)
        else:
            nc.gpsimd.tensor_copy(dst, src)

    MULT = mybir.AluOpType.mult
    ADD = mybir.AluOpType.add
    SIG = mybir.ActivationFunctionType.Sigmoid
    TANH = mybir.ActivationFunctionType.Tanh

    with tile.TileContext(nc) as tc:
        with tc.tile_pool(name="const", bufs=1) as cpool, \
             tc.tile_pool(name="stat", bufs=2) as spool, \
             tc.tile_pool(name="work", bufs=1) as wpool, \
             tc.tile_pool(name="stage", bufs=2) as gpool, \
             tc.tile_pool(name="psum", bufs=1, space="PSUM") as ppool, \
             tc.tile_pool(name="dram", bufs=1, space="DRAM") as dpool:

            # ---- resident constants (order = DMA priority: critical first) ----
            x_sb = cpool.tile([128, JT * T * C], BF16, name="x_sb", tag="x_sb")
            nc.sync.dma_start(x_sb[:], x_in[:])
            c_all = []
            for l, cin in ((0, c0_in), (1, c1_in)):
                ct = cpool.tile([128, IT * H], F32, name=f"c{l}_all", tag=f"c{l}_all")
                nc.sync.dma_start(ct[:], cin[:])
                c_all.append(ct)
            stat0 = spool.tile([128, JT * SC0], BF16, name="stat0", tag="stat0")
            nc.sync.dma_start(stat0[:], s0i_in[:])
            gt_sb = []
            for jt in range(JT):
                t_ = cpool.tile([128, K * HALF], BF16, name=f"gt{jt}", tag=f"gt{jt}")
                # src: gt_in[k, jt, p, i] -> dst cols (k, i)
                src = gt_in[:, jt, :, :].rearrange("k p i -> p k i")
                nc.sync.dma_start(t_[:].rearrange("p (k i) -> p k i", k=K), src)
                gt_sb.append(t_)
                if jt == 7:
                    w0_sb = cpool.tile([D0R, T * K * DOUT], BF16, name="w0_sb", tag="w0_sb")
                    nc.sync.dma_start(w0_sb[:], w0_in[:])
            w1_sb = cpool.tile([SC1, T * K * DOUT], BF16, name="w1_sb", tag="w1_sb")
            nc.sync.dma_start(w1_sb[:], w1_in[:])
            b1_sb = cpool.tile([128, T * DOUT], F32, name="b1_sb", tag="b1_sb")
            nc.sync.dma_start(b1_sb[:], b1_in[:])

            # supT tiles: bufs=1 slots; ones-row (layer0) set once
            supT0 = []
            for k in range(K):
                st = wpool.tile([D0R, HALF], BF16, name=f"supT0_{k}", tag=f"supT0_{k}")
                # engines need 32-aligned partition starts: set rows 64..66 to 1.0
                # (rows 64:66 are rewritten by every evacuation; row 66 persists)
                nc.vector.memset(st[64:D0R, :], 1.0)
                supT0.append(st)
            supT1 = [wpool.tile([SC1, HALF], BF16, name=f"supT1_{k}", tag=f"supT1_{k}")
                     for k in range(K)]

            # dram bounce/output buffers for the pairwise masked ReduceScatter
            # bounce[i][slot s][l] = own h_l * (slot s is partner); rs_out = partner [h0|h1]
            bounce = [dpool.tile([2, 2, 128, IT * H], BF16, name=f"bounce{i}", tag=f"bounce{i}")
                      for i in range(2)]
            rs_out = [dpool.tile([2, 128, IT * H], BF16, name=f"rso{i}", tag=f"rso{i}")
                      for i in range(2)]

            # per-core RS slot masks: mask col s = 1 if slot s is for the partner
            # (slot s of the ReduceScatter input is the contribution FOR rank s)
            mask_sb = cpool.tile([128, 2], F32, name="mask_sb", tag="mask_sb")
            nc.sync.dma_start(mask_sb[:], mask_in[:])
            mk = [mask_sb[:, 0:1], mask_sb[:, 1:2]]

            def masked_send(src_ap, tslot, l):
                """bounce[tslot][s][l] <- own h * mask_s for both slots s."""
                for sslot in range(2):
                    hm = wpool.tile([128, IT * H], BF16, name="hm", tag=f"hm{sslot}",
                                    bufs=2)
                    nc.vector.tensor_scalar_mul(hm[:], src_ap, mk[sslot])
                    nc.sync.dma_start(bounce[tslot][sslot, l], hm[:])

            # t=0: send masked init-h1 into bounce slot l=1
            h1i_sb = cpool.tile([128, IT * H], BF16, name="h1i_sb", tag="h1i_sb")
            nc.sync.dma_start(h1i_sb[:], h1b_in[:])
            masked_send(h1i_sb[:], 0, 1)

            # psum pools: einsum1 units roll through shared tags (3 banks x 2 bufs)
            def e1_alloc():
                return [ppool.tile([128, 512], F32, name=f"e1p{k}", tag=f"e1p{k}",
                                   bufs=2) for k in range(K)]

            def e1_mm(psum_k, stat, sc, rows, jlo, jhi, ih):
                for jt in range(jlo, jhi):
                    lhs = stat[:, jt * sc:(jt + 1) * sc]
                    for k in range(K):
                        nc.tensor.matmul(
                            psum_k[k][:rows, :],
                            lhs,
                            gt_sb[jt][:, k * HALF + ih * 512: k * HALF + ih * 512 + 512],
                            start=(jt == jlo), stop=(jt == jhi - 1),
                        )

            def e1_own(stat, sc, rows, supT):
                """Own-half j contraction, evacuated per ih (psum released)."""
                for ih in range(2):
                    psum_k = e1_alloc()
                    e1_mm(psum_k, stat, sc, rows, 0, 8, ih)
                    for k in range(K):
                        dst = supT[k][:rows, ih * 512:(ih + 1) * 512]
                        if k == 1:
                            nc.scalar.copy(dst, psum_k[k][:rows, :])
                        else:
                            nc.vector.tensor_copy(dst, psum_k[k][:rows, :])

            def e1_partner(stat, sc, rows, supT):
                """Partner-half j contraction, accumulated into supT (DVE add)."""
                for ih in range(2):
                    psum_k = e1_alloc()
                    e1_mm(psum_k, stat, sc, rows, 8, 16, ih)
                    for k in range(K):
                        dst = supT[k][:rows, ih * 512:(ih + 1) * 512]
                        nc.vector.tensor_tensor(dst, psum_k[k][:rows, :], dst, ADD)

            def einsum2_and_gates(t, l, supT, d_rows, w_sb, conv_all, c_t, h_dst_fn,
                                  send_fn):
                """Per half ih: conv = supT.T @ W (+bias), LSTM gates, h sends."""
                for ih in range(2):
                    for it in range(ih * 4, ih * 4 + 4):
                        pc = ppool.tile([128, DOUT], F32, name="e2p", tag="e2p", bufs=2)
                        for k in range(K):
                            nc.tensor.matmul(
                                pc[:],
                                supT[k][:d_rows, it * 128:(it + 1) * 128],
                                w_sb[:d_rows, (t * K + k) * DOUT:(t * K + k + 1) * DOUT],
                                start=(k == 0), stop=(k == K - 1),
                            )
                        dst = conv_all[:, it * DOUT:(it + 1) * DOUT]
                        if l == 0:
                            if it % 2 == 0:
                                nc.vector.tensor_copy(dst, pc[:])
                            else:
                                nc.scalar.copy(dst, pc[:])
                        else:
                            nc.vector.tensor_tensor(
                                dst, pc[:], b1_sb[:, t * DOUT:(t + 1) * DOUT], ADD)
                    # gates on this half: [128, 4*64] batched ops
                    HB = 4 * H
                    cv = conv_all[:, ih * 4 * DOUT:(ih + 1) * 4 * DOUT].rearrange(
                        "p (it g c) -> p it g c", g=4, c=H)
                    sig_i = wpool.tile([128, HB], F32, name="g_si", tag="g_si", bufs=2)
                    sig_f = wpool.tile([128, HB], F32, name="g_sf", tag="g_sf", bufs=2)
                    sig_o = wpool.tile([128, HB], F32, name="g_so", tag="g_so", bufs=2)
                    tanh_g = wpool.tile([128, HB], F32, name="g_tg", tag="g_tg", bufs=2)
                    nc.scalar.activation(sig_f[:], cv[:, :, 1, :], SIG)
                    nc.scalar.activation(sig_i[:], cv[:, :, 0, :], SIG)
                    nc.scalar.activation(tanh_g[:], cv[:, :, 3, :], TANH)
                    nc.scalar.activation(sig_o[:], cv[:, :, 2, :], SIG)
                    m1 = wpool.tile([128, HB], F32, name="g_m1", tag="g_m1", bufs=2)
                    m2 = wpool.tile([128, HB], F32, name="g_m2", tag="g_m2", bufs=2)
                    ch = c_t[:, ih * HB:(ih + 1) * HB]
                    nc.vector.tensor_tensor(m1[:], sig_f[:], ch, MULT)
                    nc.vector.tensor_tensor(m2[:], sig_i[:], tanh_g[:], MULT)
                    nc.vector.tensor_tensor(ch, m1[:], m2[:], ADD)
                    tanh_c = wpool.tile([128, HB], F32, name="g_tc", tag="g_tc", bufs=2)
                    nc.scalar.activation(tanh_c[:], ch, TANH)
                    nc.vector.tensor_tensor(h_dst_fn(ih), sig_o[:], tanh_c[:], MULT)
                    if send_fn is not None:
                        send_fn(ih, sig_o, tanh_c)

            def masked_half_send(tb, l, ih, sig_o, tanh_c):
                """bounce[tb][s][l][:, ih half] <- sig_o * mask_s * tanh_c."""
                for sslot in range(2):
                    hm = wpool.tile([128, 4 * H], BF16, name="hm", tag=f"hm{sslot}",
                                    bufs=2)
                    nc.vector.scalar_tensor_tensor(
                        hm[:], sig_o[:], mk[sslot], tanh_c[:], MULT, MULT)
                    nc.sync.dma_start(
                        bounce[tb][sslot, l][:, ih * 256:(ih + 1) * 256], hm[:])

            h1_prev = None
            last_s1v = None
            last_conv = None

            # t=0 preamble: layer-0 einsum1 entirely from the init stationary
            e1_own(stat0, SC0, DIN0, supT0)
            e1_partner(stat0, SC0, DIN0, supT0)

            for t in range(T):
                # stat1 for this step: h1-own cols written as early as possible
                stat1 = spool.tile([128, JT * SC1], BF16, name="stat1", tag="stat1")
                s1v = stat1[:].rearrange("p (jt c) -> p jt c", c=SC1)
                if t == 0:
                    nc.sync.dma_start(
                        s1v[:, 0:8, H:SC1],
                        s1i_in[:].rearrange("p (jt c) -> p jt c", c=SC1)[:, 0:8, H:SC1])
                else:
                    nc.vector.tensor_copy(
                        s1v[:, 0:8, H:SC1],
                        h1_prev[:].rearrange("p (it c) -> p it c", c=H))

                # ---------------- layer 0 step t: einsum2 + gates ----------------
                # (einsum1 for l0 step t ran during step t-1)
                conv0 = wpool.tile([128, IT * DOUT], F32, name="conv0", tag="conv0")
                einsum2_and_gates(
                    t, 0, supT0, D0R, w0_sb, conv0, c_all[0],
                    lambda ih: s1v[:, ih * 4:(ih + 1) * 4, 0:H],
                    lambda ih, so, tc_: masked_half_send(t % 2, 0, ih, so, tc_))
                h0_dst = s1v[:, 0:8, 0:H]

                nc.gpsimd.collective_compute(
                    "ReduceScatter", mybir.AluOpType.add, replica_groups=PAIRS,
                    ins=[bounce[t % 2].opt()], outs=[rs_out[t % 2].opt()],
                )

                # stat0 for step t+1: x + own h0 (ready right after l0 gates)
                stat0n = None
                if t + 1 < T:
                    stat0n = spool.tile([128, JT * SC0], BF16, name="stat0", tag="stat0")
                    s0v = stat0n[:].rearrange("p (jt c) -> p jt c", c=SC0)
                    xv = x_sb[:].rearrange("p (jt tt d) -> p jt tt d", tt=T, d=C)
                    nc.scalar.copy(s0v[:, :, H:SC0], xv[:, :, t + 1, :])
                    nc.vector.tensor_copy(s0v[:, 0:8, 0:H], h0_dst)

                # RS-window fill: own-half einsum1 of l1_t and l0_{t+1}
                e1_own(stat1, SC1, DIN1, supT1)
                if t + 1 < T:
                    e1_own(stat0n, SC0, DIN0, supT0)

                # partner halves arrive at static offsets: plain DMAs from rs_out
                nc.sync.dma_start(s1v[:, 8:16, 0:H],
                                  rs_out[t % 2][0].rearrange("p (it c) -> p it c", c=H))
                nc.scalar.dma_start(s1v[:, 8:16, H:SC1],
                                    rs_out[t % 2][1].rearrange("p (it c) -> p it c", c=H))

                # ---------------- layer 1 step t ----------------
                e1_partner(stat1, SC1, DIN1, supT1)
                conv1 = wpool.tile([128, IT * DOUT], F32, name="conv1", tag="conv1")
                h1_cur = spool.tile([128, IT * H], BF16, name="h1_all", tag="h1_all")
                h1v = h1_cur[:].rearrange("p (it c) -> p it c", c=H)
                einsum2_and_gates(
                    t, 1, supT1, SC1, w1_sb, conv1, c_all[1],
                    lambda ih: h1v[:, ih * 4:(ih + 1) * 4, :],
                    (lambda ih, so, tc_: masked_half_send((t + 1) % 2, 1, ih, so, tc_))
                    if t + 1 < T else None)

                # l0_{t+1} partner einsum1
                if t + 1 < T:
                    nc.sync.dma_start(
                        s0v[:, 8:16, 0:H],
                        rs_out[t % 2][0].rearrange("p (it c) -> p it c", c=H))
                    e1_partner(stat0n, SC0, DIN0, supT0)
                    stat0 = stat0n

                h1_prev = h1_cur
                last_s1v = s1v

            # ---------------- outputs ----------------
            hf0 = wpool.tile([128, IT * H], F32, name="hf0", tag="hf0")
            nc.vector.tensor_copy(
                hf0[:].rearrange("p (it c) -> p it c", c=H), last_s1v[:, 0:8, 0:H])
            hf1 = wpool.tile([128, IT * H], F32, name="hf1", tag="hf1")
            nc.vector.tensor_copy(hf1[:], h1_prev[:])
            nc.sync.dma_start(out_ext[0, 0], hf0[:])
            nc.sync.dma_start(out_ext[0, 1], hf1[:])
            nc.sync.dma_start(out_ext[1, 0], c_all[0][:])
            nc.sync.dma_start(out_ext[1, 1], c_all[1][:])

    nc.compile()
    _CACHE["nc"] = nc
    return nc


def _host_prep(inputs):
    """Per-core input maps."""
    G = np.asarray(inputs["G"], np.float32)
    x_seq = np.asarray(inputs["x_seq"], np.float32)
    init_h = np.asarray(inputs["init_h"], np.float32)
    init_c = np.asarray(inputs["init_c"], np.float32)
    x_meta = np.asarray(inputs["x_meta"], np.float32)

    def mlp(b, w1, b1, w2, b2):
        hid = np.maximum(x_meta[b] @ w1 + b1, 0.0)
        return hid @ w2 + b2

    in_maps = []
    for c in range(NCORES):
        b, half = c // 2, c % 2
        own = np.arange(half * HALF, (half + 1) * HALF)
        par = np.arange((1 - half) * HALF, (2 - half) * HALF)
        jperm = np.concatenate([own, par])

        # GT[k, j_local, i_own] -> [K, JT, 128, HALF]
        gt = G[:, own, :].transpose(0, 2, 1)[:, jperm, :]
        gt = np.ascontiguousarray(gt.reshape(K, JT, 128, HALF)).astype(BF)

        xl = x_seq[b][:, jperm, :]                      # [T, 2048, C]
        # x_all[p, jt, t, d]
        xa = xl.reshape(T, JT, 128, C).transpose(2, 1, 0, 3)
        xa = np.ascontiguousarray(xa.reshape(128, JT * T * C)).astype(BF)

        # stat0_init: per jt cols [h(64) | x_0(2)]
        s0 = np.zeros((128, JT, SC0), np.float32)
        h0l = init_h[0, b][jperm].reshape(JT, 128, H).transpose(1, 0, 2)
        s0[:, :, 0:H] = h0l
        s0[:, :, H:SC0] = xl[0].reshape(JT, 128, C).transpose(1, 0, 2)
        s0 = s0.reshape(128, JT * SC0).astype(BF)

        # stat1_init: per jt cols [h0(64) | h1(64)]
        s1 = np.zeros((128, JT, SC1), np.float32)
        s1[:, :, 0:H] = h0l
        s1[:, :, H:SC1] = init_h[1, b][jperm].reshape(JT, 128, H).transpose(1, 0, 2)
        s1 = s1.reshape(128, JT * SC1).astype(BF)

        c0 = np.ascontiguousarray(
            init_c[0, b][own].reshape(IT, 128, H).transpose(1, 0, 2).reshape(128, IT * H))
        c1 = np.ascontiguousarray(
            init_c[1, b][own].reshape(IT, 128, H).transpose(1, 0, 2).reshape(128, IT * H))
        h1b = init_h[1, b][own].reshape(IT, 128, H).transpose(1, 0, 2).reshape(
            128, IT * H).astype(BF)

        # layer-0 weights: rows [h(64), x(2), bias(1)] per k; bias only in k=0
        W0 = mlp(b, inputs["lw1_0"], inputs["lb1_0"], inputs["lw2_0"], inputs["lb2_0"])
        W0 = np.asarray(W0, np.float32).reshape(T, K, DIN0, DOUT)
        W0p = np.concatenate([W0[:, :, C:, :], W0[:, :, :C, :]], axis=2)  # [T,K,66,O]
        bias0 = mlp(b, inputs["bw1_0"], inputs["bb1_0"], inputs["bw2_0"], inputs["bb2_0"])
        w0 = np.zeros((T, K, D0R, DOUT), np.float32)
        w0[:, :, :DIN0, :] = W0p
        w0[:, 0, DIN0, :] = np.asarray(bias0, np.float32)
        w0 = w0.transpose(2, 0, 1, 3).reshape(D0R, T * K * DOUT).astype(BF)

        W1 = mlp(b, inputs["lw1_1"], inputs["lb1_1"], inputs["lw2_1"], inputs["lb2_1"])
        W1 = np.asarray(W1, np.float32).reshape(T, K, DIN1, DOUT)
        w1 = W1.transpose(2, 0, 1, 3).reshape(SC1, T * K * DOUT).astype(BF)
        bias1 = np.asarray(
            mlp(b, inputs["bw1_1"], inputs["bb1_1"], inputs["bw2_1"], inputs["bb2_1"]),
            np.float32)
        b1 = np.broadcast_to(bias1.reshape(1, T * DOUT), (128, T * DOUT))
        b1 = np.ascontiguousarray(b1)

        in_maps.append({
            "gt": gt,
            "x_all": xa,
            "stat0_init": s0,
            "stat1_init": s1,
            "c0_init": np.ascontiguousarray(c0, np.float32),
            "c1_init": np.ascontiguousarray(c1, np.float32),
            "h1b_init": np.ascontiguousarray(h1b),
            "w0": np.ascontiguousarray(w0),
            "w1": np.ascontiguousarray(w1),
            "bias1": b1,
            "mask": np.ascontiguousarray(np.broadcast_to(
                np.array([1 - half, half], np.float32).reshape(1, 2), (128, 2))),
        })
    return in_maps


def kernel(**inputs) -> np.ndarray:
    global LAST_RESULT
    nc = _build()
    in_maps = _host_prep(inputs)
    res = run_bass_kernel_spmd(nc, in_maps, list(range(NCORES)))
    LAST_RESULT = res

    out = np.zeros((2, L, B, N, H), np.float32)
    for c in range(NCORES):
        b, half = c // 2, c % 2
        o = res.results[c]["out"].reshape(2, L, 128, IT, H)
        # node = half*1024 + it*128 + p
        out[:, :, b, half * HALF:(half + 1) * HALF, :] = o.transpose(0, 1, 3, 2, 4).reshape(
            2, L, HALF, H)
    return out



# revision 2
# speedup vs baseline: 1.2008x; 1.2008x over previous
"""Trainium2 Bass kernel for nn_Encoder_61177514164477 (meta-GCN LSTM encoder).

Sharding: 8 cores = 4 batch groups x 2 node-halves. Core c handles batch
b = c//2 and node rows [half*1024, (half+1)*1024) with half = c%2.
G^T (bf16) stays SBUF-resident per core; one pairwise masked ReduceScatter
per timestep exchanges h0/h1 between the two halves of each batch pair.

Restructured vs the v1 kernel: layer-0's einsum1 (G @ [x_t | h0]) is gone.
G@h0_t already falls out of layer-1's einsum1 (rows 0:64 of supT1 =
G @ [h0_t | h1_{t-1}]), so layer-0's conv at step t+1 reuses supT1 rows
0:64 as its stationary. The tiny known-ahead G@x_t part is computed on the
host and shipped as a 49-row stationary (48 GX rows for all (k,t,c) plus a
ones row that carries the conv bias via the weight matrix).

Per step the PE does only:
  conv0:   8 it x (3 h-MMs contraction 64 + 1 x/bias-MM contraction 49) @N=256
  einsum1: 16 jt x 6 (k,ih) MMs @N=512, single-phase PSUM accumulation
           across all 16 j-tiles (6 banks: 3k x 2ih)
  conv1:   8 it x 3 k MMs @N=256
"""
import os
import numpy as np
import ml_dtypes

NO_GPSIMD = os.environ.get("NO_GPSIMD", "") != ""

import concourse.bass as bass
import concourse.mybir as mybir
import concourse.tile as tile
import concourse.bacc as bacc
import concourse.tile_utils as tile_utils
from concourse.bass_utils import run_bass_kernel_spmd

# use the full cayman SBUF (224 KiB phys / ~208 usable per partition)
tile_utils.max_sbuf_usage = 204 * 1024

L, B, T, N, C, H, K, M = 2, 4, 8, 2048, 2, 64, 3, 32
DIN0, DIN1, DOUT = C + H, 2 * H, 4 * H
HALF = N // 2          # 1024 rows per core
JT = N // 128          # 16 j-tiles (local order: 8 own + 8 partner)
IT = HALF // 128       # 8 own i-tiles
NCORES = 8
PAIRS = [[0, 1], [2, 3], [4, 5], [6, 7]]

F32 = mybir.dt.float32
BF16 = mybir.dt.bfloat16
BF = ml_dtypes.bfloat16

SC1 = DIN1             # 128 stationary cols per j-tile ([h0|h1])
XR = K * T * C + 1     # 49 rows of the x/bias stationary (48 GX + ones)

_CACHE = {}
LAST_RESULT = None


def _build():
    if "nc" in _CACHE:
        return _CACHE["nc"]
    nc = bacc.Bacc(None, target_bir_lowering=False, debug=False)

    gt_in = nc.declare_dram_parameter("gt", [K, JT, 128, HALF], BF16, isOutput=False)
    gxt_in = nc.declare_dram_parameter("gxt", [XR, HALF], BF16, isOutput=False)
    w0h_in = nc.declare_dram_parameter("w0h", [H, T * K * DOUT], BF16, isOutput=False)
    w0xb_in = nc.declare_dram_parameter("w0xb", [XR, T * DOUT], BF16, isOutput=False)
    w1_in = nc.declare_dram_parameter("w1", [SC1, T * K * DOUT], BF16, isOutput=False)
    b1_in = nc.declare_dram_parameter("bias1", [128, T * DOUT], F32, isOutput=False)
    supi_in = nc.declare_dram_parameter("sup_init", [H, K * HALF], BF16, isOutput=False)
    h1i_in = nc.declare_dram_parameter("h1_init", [128, IT * H], BF16, isOutput=False)
    c0_in = nc.declare_dram_parameter("c0_init", [128, IT * H], F32, isOutput=False)
    c1_in = nc.declare_dram_parameter("c1_init", [128, IT * H], F32, isOutput=False)
    mask_in = nc.declare_dram_parameter("mask", [128, 2], F32, isOutput=False)
    out_ext = nc.declare_dram_parameter("out", [2, L, 128, IT * H], F32, isOutput=True)

    MULT = mybir.AluOpType.mult
    ADD = mybir.AluOpType.add
    SIG = mybir.ActivationFunctionType.Sigmoid
    TANH = mybir.ActivationFunctionType.Tanh

    with tile.TileContext(nc) as tc:
        with tc.tile_pool(name="const", bufs=1) as cpool, \
             tc.tile_pool(name="stat", bufs=2) as spool, \
             tc.tile_pool(name="work", bufs=1) as wpool, \
             tc.tile_pool(name="psum", bufs=1, space="PSUM") as ppool, \
             tc.tile_pool(name="dram", bufs=1, space="DRAM") as dpool:

            # ---- phase-1 DMAs: everything conv0_0 + gates0_0 need ----
            gxt_sb = cpool.tile([XR, HALF], BF16, name="gxt_sb", tag="gxt_sb")
            nc.sync.dma_start(gxt_sb[:], gxt_in[:])
            w0h_sb = cpool.tile([H, T * K * DOUT], BF16, name="w0h_sb", tag="w0h_sb")
            nc.sync.dma_start(w0h_sb[:], w0h_in[:])
            w0xb_sb = cpool.tile([XR, T * DOUT], BF16, name="w0xb_sb", tag="w0xb_sb")
            nc.sync.dma_start(w0xb_sb[:], w0xb_in[:])
            # supT1 holds einsum1 output; rows 0:64 preloaded with G@h0_init
            supT1 = [wpool.tile([128, HALF], BF16, name=f"supT1_{k}", tag=f"supT1_{k}")
                     for k in range(K)]
            for k in range(K):
                nc.sync.dma_start(supT1[k][0:H, :], supi_in[:, k * HALF:(k + 1) * HALF])
            c_all = []
            for l, cin in ((0, c0_in), (1, c1_in)):
                ct = cpool.tile([128, IT * H], F32, name=f"c{l}_all", tag=f"c{l}_all")
                nc.sync.dma_start(ct[:], cin[:])
                c_all.append(ct)
            mask_sb = cpool.tile([128, 2], F32, name="mask_sb", tag="mask_sb")
            nc.sync.dma_start(mask_sb[:], mask_in[:])
            mk = [mask_sb[:, 0:1], mask_sb[:, 1:2]]
            h1i_sb = cpool.tile([128, IT * H], BF16, name="h1i_sb", tag="h1i_sb")
            nc.sync.dma_start(h1i_sb[:], h1i_in[:])

            # ---- bulk DMAs ----
            gt_sb = []
            for jt in range(JT):
                t_ = cpool.tile([128, K * HALF], BF16, name=f"gt{jt}", tag=f"gt{jt}")
                # src: gt_in[k, jt, p, i] -> dst cols (k, i)
                src = gt_in[:, jt, :, :].rearrange("k p i -> p k i")
                nc.sync.dma_start(t_[:].rearrange("p (k i) -> p k i", k=K), src)
                gt_sb.append(t_)
                if jt == 3:
                    w1_sb = cpool.tile([SC1, T * K * DOUT], BF16, name="w1_sb",
                                       tag="w1_sb")
                    nc.sync.dma_start(w1_sb[:], w1_in[:])
                if jt == 5:
                    b1_sb = cpool.tile([128, T * DOUT], F32, name="b1_sb", tag="b1_sb")
                    nc.sync.dma_start(b1_sb[:], b1_in[:])

            # dram bounce/output buffers for the pairwise masked ReduceScatter
            # bounce[i][slot s][l] = own h_l * (slot s is partner); rs_out = partner
            bounce = [dpool.tile([2, 2, 128, IT * H], BF16, name=f"bounce{i}",
                                 tag=f"bounce{i}") for i in range(2)]
            rs_out = [dpool.tile([2, 128, IT * H], BF16, name=f"rso{i}", tag=f"rso{i}")
                      for i in range(2)]

            def masked_send(src_ap, tslot, l):
                """bounce[tslot][s][l] <- own h * mask_s for both slots s."""
                for sslot in range(2):
                    hm = wpool.tile([128, IT * H], BF16, name="hm", tag=f"hm{sslot}",
                                    bufs=2)
                    nc.vector.tensor_scalar_mul(hm[:], src_ap, mk[sslot])
                    nc.sync.dma_start(bounce[tslot][sslot, l], hm[:])

            # t=0: send masked init-h1 into bounce slot l=1
            masked_send(h1i_sb[:], 0, 1)

            def conv_mms(pc, t, it, l):
                """One [128, DOUT] psum accumulation for node tile it."""
                if l == 0:
                    for k in range(K):
                        nc.tensor.matmul(
                            pc[:],
                            supT1[k][0:H, it * 128:(it + 1) * 128],
                            w0h_sb[:, (t * K + k) * DOUT:(t * K + k + 1) * DOUT],
                            start=(k == 0), stop=False)
                    nc.tensor.matmul(
                        pc[:],
                        gxt_sb[:, it * 128:(it + 1) * 128],
                        w0xb_sb[:, t * DOUT:(t + 1) * DOUT],
                        start=False, stop=True)
                else:
                    for k in range(K):
                        nc.tensor.matmul(
                            pc[:],
                            supT1[k][:, it * 128:(it + 1) * 128],
                            w1_sb[:, (t * K + k) * DOUT:(t * K + k + 1) * DOUT],
                            start=(k == 0), stop=(k == K - 1))

            def einsum2_and_gates(t, l, conv_all, c_t, h_dst_fn, send_fn):
                """Per half ih: conv psum -> sbuf, LSTM gates, h writes/sends."""
                for ih in range(2):
                    for it in range(ih * 4, ih * 4 + 4):
                        pc = ppool.tile([128, DOUT], F32, name="e2p", tag="e2p", bufs=2)
                        conv_mms(pc, t, it, l)
                        dst = conv_all[:, it * DOUT:(it + 1) * DOUT]
                        if l == 0:
                            if it % 2 == 0:
                                nc.vector.tensor_copy(dst, pc[:])
                            else:
                                nc.scalar.copy(dst, pc[:])
                        else:
                            nc.vector.tensor_tensor(
                                dst, pc[:], b1_sb[:, t * DOUT:(t + 1) * DOUT], ADD)
                    # gates on this half: [128, 4*64] batched ops
                    HB = 4 * H
                    cv = conv_all[:, ih * 4 * DOUT:(ih + 1) * 4 * DOUT].rearrange(
                        "p (it g c) -> p it g c", g=4, c=H)
                    sig_i = wpool.tile([128, HB], F32, name="g_si", tag="g_si", bufs=2)
                    sig_f = wpool.tile([128, HB], F32, name="g_sf", tag="g_sf", bufs=2)
                    sig_o = wpool.tile([128, HB], F32, name="g_so", tag="g_so", bufs=2)
                    tanh_g = wpool.tile([128, HB], F32, name="g_tg", tag="g_tg", bufs=2)
                    nc.scalar.activation(sig_f[:], cv[:, :, 1, :], SIG)
                    nc.scalar.activation(sig_i[:], cv[:, :, 0, :], SIG)
                    nc.scalar.activation(tanh_g[:], cv[:, :, 3, :], TANH)
                    nc.scalar.activation(sig_o[:], cv[:, :, 2, :], SIG)
                    m1 = wpool.tile([128, HB], F32, name="g_m1", tag="g_m1", bufs=2)
                    m2 = wpool.tile([128, HB], F32, name="g_m2", tag="g_m2", bufs=2)
                    ch = c_t[:, ih * HB:(ih + 1) * HB]
                    nc.vector.tensor_tensor(m1[:], sig_f[:], ch, MULT)
                    nc.vector.tensor_tensor(m2[:], sig_i[:], tanh_g[:], MULT)
                    nc.vector.tensor_tensor(ch, m1[:], m2[:], ADD)
                    tanh_c = wpool.tile([128, HB], F32, name="g_tc", tag="g_tc", bufs=2)
                    nc.scalar.activation(tanh_c[:], ch, TANH)
                    nc.vector.tensor_tensor(h_dst_fn(ih), sig_o[:], tanh_c[:], MULT)
                    if send_fn is not None:
                        send_fn(ih, sig_o, tanh_c)

            def masked_half_send(tb, l, ih, sig_o, tanh_c):
                """bounce[tb][s][l][:, ih half] <- sig_o * mask_s * tanh_c."""
                for sslot in range(2):
                    hm = wpool.tile([128, 4 * H], BF16, name="hm", tag=f"hm{sslot}",
                                    bufs=2)
                    nc.vector.scalar_tensor_tensor(
                        hm[:], sig_o[:], mk[sslot], tanh_c[:], MULT, MULT)
                    nc.sync.dma_start(
                        bounce[tb][sslot, l][:, ih * 256:(ih + 1) * 256], hm[:])

            stat1_next = None
            s1v_next = None
            last_s1v = None
            hf1 = wpool.tile([128, IT * H], F32, name="hf1", tag="hf1")

            for t in range(T):
                if t == 0:
                    stat1 = spool.tile([128, JT * SC1], BF16, name="stat1", tag="stat1")
                    s1v = stat1[:].rearrange("p (jt c) -> p jt c", c=SC1)
                    # own h1 init cols
                    nc.vector.tensor_copy(
                        s1v[:, 0:8, H:SC1],
                        h1i_sb[:].rearrange("p (it c) -> p it c", c=H))
                else:
                    stat1, s1v = stat1_next, s1v_next
                if t + 1 < T:
                    stat1_next = spool.tile([128, JT * SC1], BF16, name="stat1",
                                            tag="stat1")
                    s1v_next = stat1_next[:].rearrange("p (jt c) -> p jt c", c=SC1)

                # ---------------- layer 0 step t: conv0 + gates ----------------
                conv0 = wpool.tile([128, IT * DOUT], F32, name="conv0", tag="conv0")
                einsum2_and_gates(
                    t, 0, conv0, c_all[0],
                    lambda ih: s1v[:, ih * 4:(ih + 1) * 4, 0:H],
                    lambda ih, so, tc_: masked_half_send(t % 2, 0, ih, so, tc_))

                nc.gpsimd.collective_compute(
                    "ReduceScatter", mybir.AluOpType.add, replica_groups=PAIRS,
                    ins=[bounce[t % 2].opt()], outs=[rs_out[t % 2].opt()],
                )

                # ---------------- einsum1: supT1 = G^T-contract of [h0_t|h1_{t-1}]
                e1p = [[ppool.tile([128, 512], F32, name=f"e1p{k}{ih}",
                                   tag=f"e1p{k}{ih}") for ih in range(2)]
                       for k in range(K)]

                def e1_jt(jlo, jhi):
                    for jt in range(jlo, jhi):
                        lhs = stat1[:, jt * SC1:(jt + 1) * SC1]
                        for k in range(K):
                            for ih in range(2):
                                nc.tensor.matmul(
                                    e1p[k][ih][:],
                                    lhs,
                                    gt_sb[jt][:, k * HALF + ih * 512:
                                              k * HALF + ih * 512 + 512],
                                    start=(jt == 0), stop=(jt == JT - 1))

                e1_jt(0, 8)

                # partner halves arrive at static offsets: plain DMAs from rs_out
                nc.sync.dma_start(s1v[:, 8:16, 0:H],
                                  rs_out[t % 2][0].rearrange("p (it c) -> p it c", c=H))
                nc.scalar.dma_start(s1v[:, 8:16, H:SC1],
                                    rs_out[t % 2][1].rearrange("p (it c) -> p it c",
                                                               c=H))
                e1_jt(8, 16)

                # evacuate psum -> supT1 (bf16), alternating engines
                for k in range(K):
                    for ih in range(2):
                        dst = supT1[k][:, ih * 512:(ih + 1) * 512]
                        if (k + ih) % 3 == 1:
                            nc.scalar.copy(dst, e1p[k][ih][:])
                        else:
                            nc.vector.tensor_copy(dst, e1p[k][ih][:])

                # ---------------- layer 1 step t ----------------
                conv1 = wpool.tile([128, IT * DOUT], F32, name="conv1", tag="conv1")
                if t + 1 < T:
                    h1_dst = lambda ih: s1v_next[:, ih * 4:(ih + 1) * 4, H:SC1]
                    h1_send = lambda ih, so, tc_: masked_half_send(
                        (t + 1) % 2, 1, ih, so, tc_)
                else:
                    hfv = hf1[:].rearrange("p (it c) -> p it c", c=H)
                    h1_dst = lambda ih: hfv[:, ih * 4:(ih + 1) * 4, :]
                    h1_send = None
                einsum2_and_gates(t, 1, conv1, c_all[1], h1_dst, h1_send)

                last_s1v = s1v

            # ---------------- outputs ----------------
            hf0 = wpool.tile([128, IT * H], F32, name="hf0", tag="hf0")
            nc.vector.tensor_copy(
                hf0[:].rearrange("p (it c) -> p it c", c=H), last_s1v[:, 0:8, 0:H])
            nc.sync.dma_start(out_ext[0, 0], hf0[:])
            nc.sync.dma_start(out_ext[0, 1], hf1[:])
            nc.sync.dma_start(out_ext[1, 0], c_all[0][:])
            nc.sync.dma_start(out_ext[1, 1], c_all[1][:])

    nc.compile()
    _CACHE["nc"] = nc
    return nc


def _host_prep(inputs):
    """Per-core input maps."""
    G = np.asarray(inputs["G"], np.float32)
    x_seq = np.asarray(inputs["x_seq"], np.float32)
    init_h = np.asarray(inputs["init_h"], np.float32)
    init_c = np.asarray(inputs["init_c"], np.float32)
    x_meta = np.asarray(inputs["x_meta"], np.float32)

    def mlp(b, w1, b1, w2, b2):
        hid = np.maximum(x_meta[b] @ w1 + b1, 0.0)
        return hid @ w2 + b2

    # GX[b, k, t, c, i] = sum_j G[k, i, j] x_seq[b, t, j, c]   (full-N once)
    # einsum via one matmul: G [K*N, N] @ x [N, B*T*C]
    xf = x_seq.transpose(2, 0, 1, 3).reshape(N, B * T * C)
    gx = (G.reshape(K * N, N) @ xf).reshape(K, N, B, T, C)

    in_maps = []
    for c in range(NCORES):
        b, half = c // 2, c % 2
        own = np.arange(half * HALF, (half + 1) * HALF)
        par = np.arange((1 - half) * HALF, (2 - half) * HALF)
        jperm = np.concatenate([own, par])

        # GT[k, j_local, i_own] -> [K, JT, 128, HALF]
        gt = G[:, own, :].transpose(0, 2, 1)[:, jperm, :]
        gt = np.ascontiguousarray(gt.reshape(K, JT, 128, HALF)).astype(BF)

        # x/bias stationary: rows k*16 + 2t + c = GX[k,t,c,own]; row 48 = 1
        gxt = np.ones((XR, HALF), np.float32)
        gxt[:XR - 1] = gx[:, own, b].transpose(0, 2, 3, 1).reshape(XR - 1, HALF)

        # layer-0 weights
        W0 = mlp(b, inputs["lw1_0"], inputs["lb1_0"], inputs["lw2_0"], inputs["lb2_0"])
        W0 = np.asarray(W0, np.float32).reshape(T, K, DIN0, DOUT)
        bias0 = np.asarray(
            mlp(b, inputs["bw1_0"], inputs["bb1_0"], inputs["bw2_0"], inputs["bb2_0"]),
            np.float32)
        # h-part: rows = h feature (64), cols = (t, k, DOUT)
        w0h = W0[:, :, C:, :].transpose(2, 0, 1, 3).reshape(H, T * K * DOUT)
        # x/bias part: [49, T*DOUT]; rows k*16+2t+c nonzero only in col-block t
        w0xb = np.zeros((XR, T, DOUT), np.float32)
        for t in range(T):
            w0xb[np.arange(K)[:, None] * (2 * T) + 2 * t + np.arange(2)[None, :],
                 t] = W0[t, :, :C, :]
        w0xb[XR - 1] = bias0.reshape(T, DOUT)
        w0xb = w0xb.reshape(XR, T * DOUT)

        # layer-1 weights
        W1 = mlp(b, inputs["lw1_1"], inputs["lb1_1"], inputs["lw2_1"], inputs["lb2_1"])
        W1 = np.asarray(W1, np.float32).reshape(T, K, DIN1, DOUT)
        w1 = W1.transpose(2, 0, 1, 3).reshape(SC1, T * K * DOUT)
        bias1 = np.asarray(
            mlp(b, inputs["bw1_1"], inputs["bb1_1"], inputs["bw2_1"], inputs["bb2_1"]),
            np.float32)
        b1 = np.ascontiguousarray(
            np.broadcast_to(bias1.reshape(1, T * DOUT), (128, T * DOUT)))

        # supT1 rows 0:64 preload = (G[k][own] @ h0_init).T  [H, K*HALF]
        h0i = init_h[0, b]
        if np.any(h0i):
            supi = np.stack([(G[k][own] @ h0i).T for k in range(K)], 0)
        else:
            supi = np.zeros((K, H, HALF), np.float32)
        supi = supi.transpose(1, 0, 2).reshape(H, K * HALF)

        h1i = init_h[1, b][own].reshape(IT, 128, H).transpose(1, 0, 2).reshape(
            128, IT * H)
        c0 = init_c[0, b][own].reshape(IT, 128, H).transpose(1, 0, 2).reshape(
            128, IT * H)
        c1 = init_c[1, b][own].reshape(IT, 128, H).transpose(1, 0, 2).reshape(
            128, IT * H)

        in_maps.append({
            "gt": gt,
            "gxt": np.ascontiguousarray(gxt).astype(BF),
            "w0h": np.ascontiguousarray(w0h).astype(BF),
            "w0xb": np.ascontiguousarray(w0xb).astype(BF),
            "w1": np.ascontiguousarray(w1).astype(BF),
            "bias1": b1,
            "sup_init": np.ascontiguousarray(supi).astype(BF),
            "h1_init": np.ascontiguousarray(h1i).astype(BF),
            "c0_init": np.ascontiguousarray(c0, np.float32),
            "c1_init": np.ascontiguousarray(c1, np.float32),
            "mask": np.ascontiguousarray(np.broadcast_to(
                np.array([1 - half, half], np.float32).reshape(1, 2), (128, 2))),
        })
    return in_maps


def kernel(**inputs) -> np.ndarray:
    global LAST_RESULT
    nc = _build()
    in_maps = _host_prep(inputs)
    res = run_bass_kernel_spmd(nc, in_maps, list(range(NCORES)))
    LAST_RESULT = res

    out = np.zeros((2, L, B, N, H), np.float32)
    for c in range(NCORES):
        b, half = c // 2, c % 2
        o = res.results[c]["out"].reshape(2, L, 128, IT, H)
        # node = half*1024 + it*128 + p
        out[:, :, b, half * HALF:(half + 1) * HALF, :] = o.transpose(
            0, 1, 3, 2, 4).reshape(2, L, HALF, H)
    return out
